# revision 7
# baseline (speedup 1.0000x reference)
"""Trainium2 Bass kernel for NodeAttention-style pooling.

Math (from the reference, which has no nonlinearity between its two linear
layers, so they collapse):
    score[b,s,v] = x[b,s,v,:] . weff          with weff = (W2 @ W1)[0]
    (the bias terms b1@W2.T + b2 are constant over the softmax axis and
     cancel exactly in the softmax)
    w = softmax(score, axis=s)
    out[b,v,:] = sum_s w[b,s,v] * x[b,s,v,:]

Sharding: the vocab axis V=1024 is split 128-per-core across 8 cores
(softmax/pooling are independent per (b, v), so no communication).

Per-core kernel (x shard [2, 128, 128, 512] f32 = 64 MiB, streamed once):
  for each (b, v-chunk of 16):
    - DMA chunk [s=128, 16*512] (128 x 32 KiB contiguous rows)
    - scores: 16x tensor_tensor_reduce (fused mul+sum over d, one DVE pass)
    - PE-transpose scores -> [16, 128], softmax along free axis
      (DVE max, ACT exp with fused accum-sum, DVE reciprocal + scale)
    - PE-transpose weights back -> [s=128, 16] for use as matmul lhsT
    - weighted sum: 16x matmul(out[1,512] psum, lhsT=w_col[128,1],
      rhs=chunk[:,v,:]) -- contraction over s on the PE
    - psum -> sbuf staging -> one 32 KiB DMA to HBM out
"""

import numpy as np

B, S, V, D = 2, 128, 1024, 512
NCORES = 8
VS = V // NCORES  # 128 vocab entries per core
VC = 16           # vocab entries per chunk
NCHUNK = VS // VC
P = 128

_NC_CACHE = {}


def build_nc():
    import concourse.bacc as bacc
    import concourse.tile as tile
    from concourse import mybir
    from concourse.dve_ops import TENSOR_TENSOR_REDUCE

    f32 = mybir.dt.float32
    nc = bacc.Bacc(
        "TRN2",
        target_bir_lowering=False,
        debug=False,
        enable_asserts=False,
        num_devices=NCORES,
    )

    x_h = nc.dram_tensor("x", [B, S, VS, D], f32, kind="ExternalInput")
    wb_h = nc.dram_tensor("weffb", [P, D], f32, kind="ExternalInput")
    id_h = nc.dram_tensor("ident", [P, P], f32, kind="ExternalInput")
    # flat [1, VS*D] per batch so the per-chunk staging row (a single SBUF
    # partition) DMAs out as one contiguous transfer
    out_h = nc.dram_tensor("out", [B, 1, VS * D], f32, kind="ExternalOutput")
    x = x_h.ap()
    wb = wb_h.ap()
    ident = id_h.ap()
    out = out_h.ap()

    with tile.TileContext(nc) as tc:
        with (
            tc.tile_pool(name="singles", bufs=1) as singles,
            tc.tile_pool(name="chunks", bufs=3) as chunks,
            tc.tile_pool(name="scorep", bufs=2) as scorep,
            tc.tile_pool(name="smalls", bufs=4) as smalls,
            tc.tile_pool(name="stagep", bufs=2) as stagep,
            tc.tile_pool(name="pst", bufs=2, space="PSUM") as pstp,
            tc.tile_pool(name="psw", bufs=2, space="PSUM") as pswp,
            tc.tile_pool(name="pso", bufs=4, space="PSUM") as psop,
        ):
            wb_t = singles.tile([P, D], f32, name="wb_t")
            nc.sync.dma_start(out=wb_t, in_=wb)
            id_t = singles.tile([P, P], f32, name="id_t")
            nc.sync.dma_start(out=id_t, in_=ident)
            # tensor_tensor_reduce must write its elementwise product
            # somewhere; a [P,1] tile broadcast over the free dim discards it
            # without burning SBUF capacity.
            dummy = singles.tile([P, 1], f32, name="dummy")

            for b in range(B):
                for ci in range(NCHUNK):
                    v0 = ci * VC
                    chunk = chunks.tile([P, VC, D], f32, name="chunk")
                    nc.sync.dma_start(out=chunk, in_=x[b, :, v0 : v0 + VC, :])

                    sc = scorep.tile([P, VC], f32, name="sc")
                    for vl in range(VC):
                        # fused multiply + free-axis sum in one DVE pass.
                        # The custom-DVE op (not the native ISA opcode, which
                        # faults on this runtime) ships its own uop table in
                        # the NEFF. accum_out = s0 + sum(in0*in1*s1).
                        nc.vector._custom_dve(
                            TENSOR_TENSOR_REDUCE,
                            out=dummy.broadcast_to((P, D)),
                            in0=chunk[:, vl, :],
                            in1=wb_t,
                            s0=0.0,
                            s1=1.0,
                            accum_out=sc[:, vl : vl + 1],
                        )

                    # softmax over s, batched over the VC vocab rows
                    scT = pstp.tile([VC, P], f32, name="scT")
                    nc.tensor.transpose(scT, sc, id_t)
                    negmax = smalls.tile([VC, 1], f32, name="negmax")
                    nc.vector.reduce_max(
                        out=negmax, in_=scT, axis=mybir.AxisListType.X, negate=True
                    )
                    ew = smalls.tile([VC, P], f32, name="ew")
                    lsum = smalls.tile([VC, 1], f32, name="lsum")
                    nc.scalar.activation(
                        out=ew,
                        in_=scT,
                        func=mybir.ActivationFunctionType.Exp,
                        bias=negmax,
                        scale=1.0,
                        accum_out=lsum,
                    )
                    rec = smalls.tile([VC, 1], f32, name="rec")
                    nc.vector.reciprocal(rec, lsum)
                    wnorm = smalls.tile([VC, P], f32, name="wnorm")
                    nc.vector.tensor_scalar_mul(wnorm, ew, rec)

                    wT = pswp.tile([P, VC], f32, name="wT")
                    nc.tensor.transpose(wT, wnorm, id_t[:VC, :VC])
                    wTs = smalls.tile([P, VC], f32, name="wTs")
                    nc.scalar.copy(wTs, wT)

                    stag = stagep.tile([1, VC * D], f32, name="stag")
                    for vl in range(VC):
                        po = psop.tile([1, D], f32, name="po")
                        nc.tensor.matmul(po, lhsT=wTs[:, vl : vl + 1], rhs=chunk[:, vl, :])
                        # PSUM is not DMA-reachable; ACT (otherwise idle)
                        # moves each result row into the SBUF staging tile.
                        # Compute engines can only start at partition 0/32/64/96,
                        # so the staging tile lives on one partition.
                        nc.scalar.copy(stag[0:1, vl * D : (vl + 1) * D], po)
                    nc.sync.dma_start(
                        out=out[b, :, v0 * D : (v0 + VC) * D], in_=stag
                    )

    nc.compile()
    return nc


def _get_nc():
    if "nc" not in _NC_CACHE:
        _NC_CACHE["nc"] = build_nc()
    return _NC_CACHE["nc"]


def _host_prep(x, W1, b1, W2, b2):
    x = np.ascontiguousarray(np.asarray(x, dtype=np.float32))
    W1 = np.asarray(W1, dtype=np.float64)
    W2 = np.asarray(W2, dtype=np.float64)
    weff = (W2 @ W1)[0].astype(np.float32)  # [D]
    weffb = np.ascontiguousarray(np.broadcast_to(weff, (P, D)))
    ident = np.eye(P, dtype=np.float32)
    in_maps = []
    for c in range(NCORES):
        shard = np.ascontiguousarray(x[:, :, c * VS : (c + 1) * VS, :])
        in_maps.append({"x": shard, "weffb": weffb, "ident": ident})
    return in_maps


def kernel(x, W1, b1, W2, b2):
    from concourse.bass_utils import run_bass_kernel_spmd

    in_maps = _host_prep(x, W1, b1, W2, b2)
    nc = _get_nc()
    res = run_bass_kernel_spmd(nc, in_maps, core_ids=list(range(NCORES)))
    out = np.concatenate(
        [r["out"].reshape(B, VS, D) for r in res.results], axis=1
    )
    return out


# revision 14
# speedup vs baseline: 1.3820x; 1.3820x over previous
"""Trainium2 Bass kernel for NodeAttention-style pooling.

Math (the reference's two linear layers have no nonlinearity between them,
so they collapse):
    score[b,s,v] = x[b,s,v,:] . weff          with weff = (W2 @ W1)[0]
    (bias terms b1@W2.T + b2 are constant over the softmax axis and cancel)
    w = softmax(score, axis=s)
    out[b,v,:] = sum_s w[b,s,v] * x[b,s,v,:]

Sharding: vocab axis V=1024 split 128-per-core across 8 cores (softmax and
pooling are independent per (b, v) — no communication).

Per-core pipeline (x shard [2, 128, 128, 512] f32 = 64 MiB streamed once,
target = HBM roofline ~186 us):
  for each (b, v-chunk of 16):
    - DMA chunk [s=128, 16*512] f32 (128 x 32 KiB contiguous rows)
    - scores: 16x custom-DVE TENSOR_TENSOR_REDUCE (fused mul + sum over d,
      one 1x DVE pass; the native ISA opcode faults on this runtime)
    - ACT converts the chunk to fp16 (for the weighted-sum matmuls; fp32
      matmul is 4 cyc/row vs fp16's 1, and f32r faults on this runtime)
    - PE-transpose scores -> [16, 128]; softmax along free axis (DVE max,
      ACT exp with fused accum-sum, DVE reciprocal + scale); PE-transpose
      weights back and convert to fp16 -> lhsT columns
    - weighted sum: 16x fp16 matmul(out[1,512] psum, lhsT=w_col[128,1],
      rhs=chunk16[:,v,:]), packed 4-per-PSUM-bank via tile_position
      col-groups (output partitions 0/32/64/96)
    - 4x ACT copies [97,512] psum->SBUF staging (junk rows included -
      engines cannot stride partitions), one strided DMA -> HBM out
"""

import numpy as np

B, S, V, D = 2, 128, 1024, 512
NCORES = 8
VS = V // NCORES  # 128 vocab entries per core
VC = 16           # vocab entries per chunk
NCHUNK = VS // VC
NGRP = VC // 4    # psum col-group packs per chunk
P = 128

_NC_CACHE = {}


def build_nc():
    import concourse.bacc as bacc
    import concourse.tile as tile
    from concourse import mybir
    from concourse.dve_ops import TENSOR_TENSOR_REDUCE

    f32 = mybir.dt.float32
    f16 = mybir.dt.float16
    nc = bacc.Bacc(
        "TRN2",
        target_bir_lowering=False,
        debug=False,
        enable_asserts=False,
        num_devices=NCORES,
    )

    x_h = nc.dram_tensor("x", [B, S, VS, D], f32, kind="ExternalInput")
    wb_h = nc.dram_tensor("weffb", [P, D], f32, kind="ExternalInput")
    id_h = nc.dram_tensor("ident", [P, P], f32, kind="ExternalInput")
    # flat [1, VS*D] per batch; the staging layout maps v = 4k+j to
    # (partition 32j, free segment k)
    out_h = nc.dram_tensor("out", [B, 1, VS * D], f32, kind="ExternalOutput")
    x = x_h.ap()
    wb = wb_h.ap()
    ident = id_h.ap()
    out = out_h.ap()

    with tile.TileContext(nc) as tc:
        with (
            tc.tile_pool(name="singles", bufs=1) as singles,
            tc.tile_pool(name="chunks", bufs=3) as chunks,
            tc.tile_pool(name="chunk16p", bufs=2) as chunk16p,
            tc.tile_pool(name="scorep", bufs=2) as scorep,
            tc.tile_pool(name="smalls", bufs=4) as smalls,
            tc.tile_pool(name="stagep", bufs=2) as stagep,
            tc.tile_pool(name="pst", bufs=2, space="PSUM") as pstp,
            tc.tile_pool(name="psw", bufs=2, space="PSUM") as pswp,
            tc.tile_pool(name="bankp", bufs=1, space="PSUM") as bankp,
        ):
            wb_t = singles.tile([P, D], f32, name="wb_t")
            nc.sync.dma_start(out=wb_t, in_=wb)
            id_t = singles.tile([P, P], f32, name="id_t")
            nc.sync.dma_start(out=id_t, in_=ident)
            # TENSOR_TENSOR_REDUCE must write its elementwise product
            # somewhere; a [P,1] tile broadcast over the free dim discards it.
            dummy = singles.tile([P, 1], f32, name="dummy")

            # One persistent 4-bank PSUM tile for the weighted-sum outputs.
            # Matmul (grp, j) writes [1,512] at (partition 32j, bank grp);
            # one ACT copy per chunk then moves partitions 0..96 (junk rows
            # included — engines cannot stride partitions) to staging. A
            # long-lived tensor (not pool-rotated) so the junk-row reads
            # don't race released tiles; zeroed once at startup.
            bigbank = bankp.tile([P, NGRP, D], f32, name="bigbank")
            nc.vector.memset(bigbank, 0.0)
            for b in range(B):
                for ci in range(NCHUNK):
                    v0 = ci * VC
                    chunk = chunks.tile([P, VC, D], f32, name="chunk")
                    nc.sync.dma_start(out=chunk, in_=x[b, :, v0 : v0 + VC, :])

                    sc = scorep.tile([P, VC], f32, name="sc")
                    for vl in range(VC):
                        # accum_out = s0 + sum(in0*in1*s1): fused dot with weff
                        nc.vector._custom_dve(
                            TENSOR_TENSOR_REDUCE,
                            out=dummy.broadcast_to((P, D)),
                            in0=chunk[:, vl, :],
                            in1=wb_t,
                            s0=0.0,
                            s1=1.0,
                            accum_out=sc[:, vl : vl + 1],
                        )

                    # fp16 copy of x for the weighted-sum matmuls (ACT has
                    # slack; DVE is busy with the score pass)
                    chunk16 = chunk16p.tile([P, VC, D], f16, name="chunk16")
                    nc.scalar.copy(chunk16, chunk)

                    # softmax over s, batched over the VC vocab rows
                    scT = pstp.tile([VC, P], f32, name="scT")
                    nc.tensor.transpose(scT, sc, id_t)
                    negmax = smalls.tile([VC, 1], f32, name="negmax")
                    nc.vector.reduce_max(
                        out=negmax, in_=scT, axis=mybir.AxisListType.X, negate=True
                    )
                    ew = smalls.tile([VC, P], f32, name="ew")
                    lsum = smalls.tile([VC, 1], f32, name="lsum")
                    nc.scalar.activation(
                        out=ew,
                        in_=scT,
                        func=mybir.ActivationFunctionType.Exp,
                        bias=negmax,
                        scale=1.0,
                        accum_out=lsum,
                    )
                    rec = smalls.tile([VC, 1], f32, name="rec")
                    nc.vector.reciprocal(rec, lsum)
                    wnorm = smalls.tile([VC, P], f32, name="wnorm")
                    # normalize on ACT (per-partition scale AP); DVE is the
                    # bottleneck engine, ACT has slack
                    nc.scalar.mul(wnorm, ew, rec)

                    wT = pswp.tile([P, VC], f32, name="wT")
                    nc.tensor.transpose(wT, wnorm, id_t[:VC, :VC])
                    wTs = smalls.tile([P, VC], f16, name="wTs")
                    nc.scalar.copy(wTs, wT)

                    # weighted sum: v = 4*grp + j -> psum bank `grp`,
                    # partition 32*j (PE col-group); then one ACT copy per
                    # bank into the staging block (junk rows 1..31 etc are
                    # copied too and simply not DMA'd out)
                    stag = stagep.tile([P, NGRP * D], f32, name="stag")
                    for grp in range(NGRP):
                        for j in range(4):
                            vl = grp * 4 + j
                            nc.tensor.matmul(
                                bigbank[32 * j : 32 * j + 1, grp, :],
                                lhsT=wTs[:, vl : vl + 1],
                                rhs=chunk16[:, vl, :],
                                tile_position=(0, 32 * j),
                            )
                    nc.scalar.copy(
                        stag[0:97, :],
                        bigbank[0:97, :, :].rearrange("p g d -> p (g d)"),
                    )
                    src = stag.rearrange("(g r) n -> g r n", r=32)[:, 0, :].rearrange(
                        "j (k d) -> j k d", d=D
                    )
                    dst = out[b, :, v0 * D : (v0 + VC) * D].rearrange(
                        "o (k j d) -> o j k d", j=4, d=D
                    )[0]
                    nc.sync.dma_start(out=dst, in_=src)

    nc.compile()
    return nc


def _get_nc():
    if "nc" not in _NC_CACHE:
        _NC_CACHE["nc"] = build_nc()
    return _NC_CACHE["nc"]


def _host_prep(x, W1, b1, W2, b2):
    x = np.ascontiguousarray(np.asarray(x, dtype=np.float32))
    W1 = np.asarray(W1, dtype=np.float64)
    W2 = np.asarray(W2, dtype=np.float64)
    weff = (W2 @ W1)[0].astype(np.float32)  # [D]
    weffb = np.ascontiguousarray(np.broadcast_to(weff, (P, D)))
    ident = np.eye(P, dtype=np.float32)
    in_maps = []
    for c in range(NCORES):
        shard = np.ascontiguousarray(x[:, :, c * VS : (c + 1) * VS, :])
        in_maps.append({"x": shard, "weffb": weffb, "ident": ident})
    return in_maps


def kernel(x, W1, b1, W2, b2):
    from concourse.bass_utils import run_bass_kernel_spmd

    in_maps = _host_prep(x, W1, b1, W2, b2)
    nc = _get_nc()
    res = run_bass_kernel_spmd(nc, in_maps, core_ids=list(range(NCORES)))
    out = np.concatenate(
        [r["out"].reshape(B, VS, D) for r in res.results], axis=1
    )
    return out
